# revision 20
# baseline (speedup 1.0000x reference)
"""Trainium2 Bass kernel for hypergraph-attention message passing.

Reference computation (per hyperedge e with members m):
    xg      = X[member_idx]                       # [T, C] gather
    h       = leaky_relu(xg @ W1 + b1)            # [T, H]
    s       = h @ W2 + b2                         # [T]
    beta    = segment_softmax(s)                  # [T], per-edge groups of 16
    Z       = segment_sum(beta * xg)              # [E, C]
    out     = tanh(leaky_relu(Z)), beta

Sharding: hyperedges split across 8 cores (6250 edges / 100000 memberships
per core); X and MLP params replicated. All compute per edge is core-local.

Per-core device pipeline (supertile ST = 16 tiles = 2048 memberships = 128
edges; membership tile = 128 consecutive memberships on partitions):
  1. indirect-DMA gathers of X rows -> xg [128, 16*128] f32 (128 rows/instr)
  2. PE transpose xg tiles -> xgT (PSUM), ACT copies PSUM->SBUF
  3. PE matmul h = xgT.T @ W1'' where W1'' = W1 * |W2| col-sorted by sign(W2)
  4. DVE relu + signed free-dim reduces -> s  (relu split of leaky-relu)
  5. ACT exp -> ex; PE matmul with 0/1 mask -> per-edge denom; DVE reciprocal
  6. PE matmul mask^T @ recip -> per-membership 1/denom; DVE mul -> beta
  7. DVE betamask = mask * beta; PE matmul xg.T @ betamask -> Z^T in PSUM
  8. DVE leaky-relu, ACT tanh, PE transpose back, DMA Z rows + beta rows out

b2 is ignored (softmax is shift-invariant so it provably cancels).
b1 is folded in only when nonzero (build-time decision from host values).
"""

import numpy as np

import concourse.bacc as bacc
import concourse.mybir as mybir
import concourse.tile as tile
from concourse.bass import IndirectOffsetOnAxis

F32 = mybir.dt.float32
BF16 = mybir.dt.bfloat16
I32 = mybir.dt.int32

C = 128          # hidden channels (X feature dim)
H = 64           # MLP hidden size
M = 16           # members per hyperedge
NEG = 0.01       # leaky relu slope
P = 128          # partitions / tile size
ST = 16          # tiles per supertile

N_NODES = 100000
E_TOT = 50000
T_TOT = E_TOT * M
N_CORES = 8
E_LOC = E_TOT // N_CORES          # 6250 edges per core
T_LOC = E_LOC * M                 # 100000 memberships per core


def _ceil_div(a, b):
    return (a + b - 1) // b


def build_nc(n_nodes, n_tiles, n_p, use_b1=False, score_bf16=False,
             xg_bufs=4, exp_bias=0.0, repeats=1):
    """Build the Bass program. n_tiles membership tiles of 128; first n_p
    columns of W1'' belong to the positive-sign group of W2.

    Score math: s = 0.99*(sum_P relu(u) - sum_N relu(u) + s_lin') + const,
    where u = xg @ W1'' (+b1''), W1'' = W1*|W2| sign-sorted, s_lin' is the
    65th matmul column (xg @ W1 @ W2 * 0.01/0.99), const lands in exp bias.
    This equals the lrelu-MLP score exactly (lrelu(x)=0.01x+0.99relu(x) and
    lrelu commutes with positive scales)."""
    nc = bacc.Bacc("TRN2", target_bir_lowering=False, debug=False)
    sdt = BF16 if score_bf16 else F32

    n_st = _ceil_div(n_tiles, ST)
    HA = H + 1  # W1'' columns + linear-term column

    x_d = nc.dram_tensor("x_nodes", [n_nodes, C], F32, kind="ExternalInput")
    idx_d = nc.dram_tensor("idx_mat", [P, n_tiles], I32, kind="ExternalInput")
    w1_d = nc.dram_tensor("w1pp", [C, HA], sdt, kind="ExternalInput")
    ident_d = nc.dram_tensor("ident", [P, P], F32, kind="ExternalInput")
    m01_d = nc.dram_tensor("mask01", [P, 8], F32, kind="ExternalInput")
    m01t_d = nc.dram_tensor("mask01t", [8, P], F32, kind="ExternalInput")
    b1_d = nc.dram_tensor("b1bcast", [P, H], F32, kind="ExternalInput")

    z_d = nc.dram_tensor("z_out", [n_st * P, C], F32, kind="ExternalOutput")
    beta_d = nc.dram_tensor("beta_out", [n_tiles, P], F32,
                            kind="ExternalOutput")

    ax = mybir.AluOpType
    act = mybir.ActivationFunctionType

    with tile.TileContext(nc) as tc:
        with (
            tc.tile_pool(name="const", bufs=1) as cpool,
            tc.tile_pool(name="xg", bufs=xg_bufs) as xgp,
            tc.tile_pool(name="xgt_ps", bufs=2, space="PSUM") as tpsp,
            tc.tile_pool(name="xgt_sb", bufs=2) as xgtp,
            tc.tile_pool(name="h_ps", bufs=2, space="PSUM") as hpsp,
            tc.tile_pool(name="hl_sb", bufs=2) as hlp,
            tc.tile_pool(name="sml", bufs=2) as smlp,
            tc.tile_pool(name="small_ps", bufs=1, space="PSUM") as spsp,
            tc.tile_pool(name="zt_ps", bufs=1, space="PSUM") as ztpsp,
            tc.tile_pool(name="zf_ps", bufs=1, space="PSUM") as zfpsp,
            tc.tile_pool(name="zl_sb", bufs=2) as zlp,
        ):
            ident = cpool.tile([P, P], F32)
            w1 = cpool.tile([C, HA], sdt)
            m01 = cpool.tile([P, 8], F32)
            m01t = cpool.tile([8, P], F32)
            idx_sb = cpool.tile([P, n_tiles], I32)
            nc.sync.dma_start(ident[:], ident_d[:])
            nc.sync.dma_start(w1[:], w1_d[:])
            nc.sync.dma_start(m01[:], m01_d[:])
            nc.sync.dma_start(m01t[:], m01t_d[:])
            nc.sync.dma_start(idx_sb[:], idx_d[:])
            if use_b1:
                b1b = cpool.tile([P, H], F32)
                nc.sync.dma_start(b1b[:], b1_d[:])
            if exp_bias != 0.0:
                expb = cpool.tile([P, 1], F32)
                nc.vector.memset(expb[:], float(exp_bias))
                exp_bias_ap = expb[:, 0:1]
            else:
                exp_bias_ap = 0.0

            def emit_st(st):
                nt = min(ST, n_tiles - st * ST)
                ne = 8 * nt

                # gather: HW indirect DMA supports exactly one offset per
                # dest partition, so one [128,1] gather per membership tile
                xg = xgp.tile([P, ST * P], F32, tag="xg")
                for k in range(nt):
                    j = st * ST + k
                    nc.gpsimd.indirect_dma_start(
                        out=xg[:, k * P : (k + 1) * P],
                        out_offset=None,
                        in_=x_d[:],
                        in_offset=IndirectOffsetOnAxis(
                            ap=idx_sb[:, j : j + 1], axis=0
                        ),
                    )

                # --- per 4-tile batch: transpose, copy, h-matmul, relu,
                #     signed reduces ---
                sp = smlp.tile([P, ST], F32, tag="sp")
                sn = smlp.tile([P, ST], F32, tag="sn")
                s_sb = smlp.tile([P, ST], F32, tag="s")
                for b in range(_ceil_div(nt, 4)):
                    nb = min(4, nt - b * 4)
                    tps = tpsp.tile([P, 4 * P], F32, tag="tps")
                    for j in range(nb):
                        k = b * 4 + j
                        nc.tensor.transpose(
                            out=tps[:, j * P : (j + 1) * P],
                            in_=xg[:, k * P : (k + 1) * P],
                            identity=ident[:],
                        )
                    xgt4 = xgtp.tile([P, 4 * P], sdt, tag="xgt")
                    nc.scalar.copy(out=xgt4[:, : nb * P], in_=tps[:, : nb * P])

                    hbank = hpsp.tile([P, 4, HA], F32, tag="hbank")
                    for j in range(nb):
                        nc.tensor.matmul(
                            out=hbank[:, j, :],
                            lhsT=xgt4[:, j * P : (j + 1) * P],
                            rhs=w1[:],
                            start=True,
                            stop=True,
                        )
                    if use_b1:
                        hsrc = hlp.tile([P, 4, H], F32, tag="hb1")
                        nc.vector.scalar_tensor_tensor(
                            out=hsrc[:, :nb, :], in0=hbank[:, :nb, :H],
                            scalar=1.0,
                            in1=b1b[:].unsqueeze(1).to_broadcast([P, nb, H]),
                            op0=ax.mult, op1=ax.add,
                        )
                        hview = hsrc[:, :nb, :]
                    else:
                        hview = hbank[:, :nb, :H]
                    hr = hlp.tile([P, 4, H], F32, tag="hr")
                    nc.vector.tensor_scalar(hr[:, :nb, :], hview, 0.0, None,
                                            ax.max)
                    c0 = b * 4
                    sl = slice(c0, c0 + nb)
                    if n_p > 0:
                        nc.vector.tensor_reduce(
                            out=sp[:, sl], in_=hr[:, :nb, :n_p],
                            axis=mybir.AxisListType.X, op=ax.add,
                        )
                    if n_p < H:
                        nc.vector.tensor_reduce(
                            out=sn[:, sl], in_=hr[:, :nb, n_p:],
                            axis=mybir.AxisListType.X, op=ax.add,
                        )
                    # s = (sP - sN) + s_lin'  (x0.99 + const fold into exp)
                    if n_p == H:
                        s0 = sp
                    elif n_p == 0:
                        nc.vector.tensor_scalar_mul(s_sb[:, sl], sn[:, sl],
                                                    -1.0)
                        s0 = s_sb
                    else:
                        nc.vector.tensor_tensor(out=s_sb[:, sl],
                                                in0=sp[:, sl],
                                                in1=sn[:, sl],
                                                op=ax.subtract)
                        s0 = s_sb
                    nc.vector.tensor_tensor(
                        out=s_sb[:, sl], in0=s0[:, sl],
                        in1=hbank[:, :nb, H], op=ax.add,
                    )

                # --- softmax pieces ---
                ex = smlp.tile([P, ST], F32, tag="ex")
                nc.scalar.activation(out=ex[:, :nt], in_=s_sb[:, :nt],
                                     func=act.Exp, scale=1.0 - NEG,
                                     bias=exp_bias_ap)
                sps = spsp.tile([P, 2 * ST], F32, tag="smallps")
                nc.tensor.matmul(out=sps[:8, :nt], lhsT=m01[:],
                                 rhs=ex[:, :nt], start=True, stop=True)
                rcp = smlp.tile([8, ST], F32, tag="rcp")
                nc.vector.reciprocal(out=rcp[:, :nt], in_=sps[:8, :nt])
                nc.tensor.matmul(out=sps[:, ST : ST + nt], lhsT=m01t[:],
                                 rhs=rcp[:, :nt], start=True, stop=True)
                beta_sb = smlp.tile([P, ST], F32, tag="beta")
                nc.vector.tensor_tensor(out=beta_sb[:, :nt], in0=ex[:, :nt],
                                        in1=sps[:, ST : ST + nt], op=ax.mult)

                # --- weighted reduce: Z^T accumulation via masked matmuls ---
                zt = ztpsp.tile([P, ST, 8], F32, tag="zt")
                for b8 in range(_ceil_div(nt, 8)):
                    n8 = min(8, nt - b8 * 8)
                    bm = smlp.tile([P, 8, 8], F32, tag="bm")
                    nc.vector.tensor_tensor(
                        out=bm[:, :n8, :],
                        in0=m01[:].unsqueeze(1).to_broadcast([P, n8, 8]),
                        in1=beta_sb[:, b8 * 8 : b8 * 8 + n8]
                            .unsqueeze(2).to_broadcast([P, n8, 8]),
                        op=ax.mult,
                    )
                    for j in range(n8):
                        k = b8 * 8 + j
                        nc.tensor.matmul(
                            out=zt[:, k, :],
                            lhsT=xg[:, k * P : (k + 1) * P],
                            rhs=bm[:, j, :],
                            start=True, stop=True,
                        )

                # lrelu(zt) = 0.01*zt + 0.99*relu(zt): single-PSUM-input ops
                zl1 = zlp.tile([P, ST, 8], F32, tag="zl1")
                nc.vector.tensor_scalar(zl1[:, :nt, :], zt[:, :nt, :],
                                        0.0, 1.0 - NEG, ax.max, ax.mult)
                zl = zlp.tile([P, ST, 8], F32, tag="zl")
                nc.vector.scalar_tensor_tensor(
                    out=zl[:, :nt, :], in0=zt[:, :nt, :], scalar=NEG,
                    in1=zl1[:, :nt, :], op0=ax.mult, op1=ax.add,
                )
                zth = zlp.tile([P, ST, 8], F32, tag="zth")
                nc.scalar.activation(out=zth[:, :nt, :], in_=zl[:, :nt, :],
                                     func=act.Tanh)
                zf = zfpsp.tile([P, 2 * P], F32, tag="zf")
                nc.tensor.transpose(out=zf[:ne, :P], in_=zth[:, :nt, :],
                                    identity=ident[:])
                zo = zlp.tile([P, P], F32, tag="zo")
                nc.vector.tensor_copy(out=zo[:ne, :], in_=zf[:ne, :P])
                nc.sync.dma_start(z_d[st * P : st * P + ne, :], zo[:ne, :])

                # --- beta out: transpose to row-major and store ---
                nc.tensor.transpose(out=zf[:nt, P:], in_=beta_sb[:, :nt],
                                    identity=ident[:])
                bo = zlp.tile([ST, P], F32, tag="bo")
                nc.vector.tensor_copy(out=bo[:nt, :], in_=zf[:nt, P:])
                nc.sync.dma_start(beta_d[st * ST : st * ST + nt, :],
                                  bo[:nt, :])

            def st_loop():
                for st in range(n_st):
                    emit_st(st)

            if repeats == 1:
                st_loop()
            else:
                with tc.For_i(0, repeats, 1):
                    st_loop()

    nc.compile()
    return nc


def _host_prep(X, member_idx, W1, b1, W2, score_bf16=False):
    """Build per-core input maps + build-time constants."""
    import ml_dtypes

    X = np.ascontiguousarray(np.asarray(X, dtype=np.float32))
    member_idx = np.asarray(member_idx).astype(np.int32)
    W1 = np.asarray(W1, dtype=np.float32)
    W2 = np.asarray(W2, dtype=np.float32).reshape(-1)
    b1 = np.asarray(b1, dtype=np.float32).reshape(-1)

    # fold |W2| into W1 columns; sort positive-sign group first; append the
    # linear-term column (x @ W1 @ W2) * NEG/(1-NEG) for the relu split
    order = np.argsort(W2 < 0, kind="stable")
    n_p = int((W2 >= 0).sum())
    w1pp = (W1 * np.abs(W2)[None, :])[:, order]
    b1s = (b1 * np.abs(W2))[order]
    w_lin = (W1 @ W2.reshape(-1, 1)) * (NEG / (1.0 - NEG))
    w1aug = np.concatenate([w1pp, w_lin], axis=1)
    exp_bias = float(NEG * (b1 @ W2.reshape(-1)))
    sdt = ml_dtypes.bfloat16 if score_bf16 else np.float32
    w1aug = np.ascontiguousarray(w1aug.astype(sdt))

    ident = np.eye(P, dtype=np.float32)
    m01 = np.zeros((P, 8), np.float32)
    m01[np.arange(P), np.arange(P) // M] = 1.0
    m01t = np.ascontiguousarray(m01.T)
    b1b = np.ascontiguousarray(np.broadcast_to(b1s, (P, H)))
    use_b1 = bool(np.any(b1s != 0.0))

    n_tiles = _ceil_div(T_LOC, P)
    t_pad = n_tiles * P
    in_maps = []
    for c in range(N_CORES):
        sl = member_idx[c * T_LOC : (c + 1) * T_LOC]
        idx = np.zeros(t_pad, np.int32)
        idx[:T_LOC] = sl
        idx_mat = np.ascontiguousarray(idx.reshape(n_tiles, P).T)
        in_maps.append({
            "x_nodes": X,
            "idx_mat": idx_mat,
            "w1pp": w1aug,
            "ident": ident,
            "mask01": m01,
            "mask01t": m01t,
            "b1bcast": b1b,
        })
    return in_maps, n_tiles, n_p, use_b1, exp_bias


def kernel(X, member_idx, segment_ids, W1, b1, W2, b2, *,
           score_bf16=False, _trace=False, _repeats=1, _nc=None,
           _in_maps=None):
    from concourse.bass_utils import run_bass_kernel_spmd

    if _in_maps is None:
        _in_maps, n_tiles, n_p, use_b1, exp_bias = _host_prep(
            X, member_idx, W1, b1, W2, score_bf16=score_bf16)
        if _nc is None:
            _nc = build_nc(N_NODES, n_tiles, n_p, use_b1=use_b1,
                           score_bf16=score_bf16, exp_bias=exp_bias,
                           repeats=_repeats)
    res = run_bass_kernel_spmd(_nc, _in_maps, core_ids=list(range(N_CORES)),
                               trace=_trace)
    z_parts, b_parts = [], []
    for c in range(N_CORES):
        z_parts.append(res.results[c]["z_out"][:E_LOC, :])
        b_parts.append(res.results[c]["beta_out"].reshape(-1)[:T_LOC])
    Z = np.concatenate(z_parts, axis=0)
    beta = np.concatenate(b_parts, axis=0)
    kernel._last_result = res
    return Z, beta
